# revision 40
# baseline (speedup 1.0000x reference)
"""Trainium2 Bass kernel for KG-enhanced embedding model (gnn_message_passing).

Computes, for full inputs:
    inputs_embeds = word_embedding[input_ids]                       # [B,S,H] gather
    h   = relu(entity_embeddings @ W1 + b1)                         # [B,E,MLP_HID]
    ent = h @ W2 + b2                                               # [B,E,H]
    out = inputs_embeds + einsum('bes,beh->bsh', entity_mask, ent)  # masked scatter-add

Sharding: data-parallel over batch B=32 -> 4 examples per core on 8 cores.
Weights and the vocab table are replicated; the gather reads only the rows
each core needs via indirect DMA (16 x 128-row gathers per core).

Numerics: the accuracy gate is rel_err < 2e-2, so everything runs in plain
bf16 (vocab table pre-cast to bf16 on host, bf16 matmuls with f32 PSUM
accumulation, bf16 output store upcast to f32 on host). Measured end-to-end
relative error ~5.8e-3. (fp8 e4m3 for the gather table also passes at
7.3e-3 but measured slower: the gather is descriptor-emission-paced, not
byte-paced, so halving row bytes buys nothing and starves the weights.)

Schedule notes (TRN2-specific, from perfetto traces):
- The per-instruction SWDGE descriptor generation on gpsimd (~1.4us per
  128-row indirect DMA, emission-paced) is the kernel's pacer: 16 x 1.4us
  is the wall everything else must hide behind. ids ride the sync HWDGE
  so gather desc-gen starts right after the framework preamble; W2's
  second half rides the gpsimd queue ahead of the gathers (its desc-gen
  overlaps the idle ids wait, its data lands before the contention).
- Under SDMA round-robin contention, a queue's bandwidth share scales
  with its descriptor size, so weight loads must use fat descriptors:
  W1 is loaded via DMA-transpose (contiguous M2S reads), W2 in two
  [128, 4*768] chunks (6KB/partition descriptors). Nothing MLP-critical
  goes on the scalar queue (it gets starved during the gather window).
- b1 is folded into the MLP1 contraction (K padded to 128 with a b1|ones
  row), b2 enters as a trailing K=1 ones-matmul: no bias-load can stall
  the MLP, which must finish before the add/store tail starts.

Shapes (hardcoded): V=30522, H=768, B=32, S=512, E=8, KG=100, MH=1000.
"""

import os
import numpy as np
from contextlib import ExitStack

V, H = 30522, 768
B, S, E = 32, 512, 8
KG, MH = 100, 1000
NCORES = 8
BPC = B // NCORES              # examples per core = 4
TOK = BPC * S                  # tokens per core = 2048
NCH = TOK // 128               # 128-token chunks per core = 16
KCH = 8                        # K chunks of 128 for the 1024-dim (padded) contraction
NE = BPC * E                   # entities per core = 32

_PROGRAM = None


def _maybe_enable_profiling():
    """Optional NTFF profiling (KERNEL_PROFILE=1): shim antenv.axon_hooks."""
    if os.environ.get("KERNEL_PROFILE") != "1":
        return False
    import sys, types
    try:
        from antenv.axon_hooks import get_axon_ntff_profile_hook  # noqa: F401
        return True
    except ImportError:
        pass
    try:
        from trn_agent_boot.trn_boot import _ntff_profile_via_ctypes
        import antenv
        hook = _ntff_profile_via_ctypes("/opt/axon/libaxon_pjrt.so")
        m = types.ModuleType("antenv.axon_hooks")
        m.get_axon_ntff_profile_hook = lambda: hook
        m.set_axon_ntff_profile_hook = lambda h: None
        sys.modules["antenv.axon_hooks"] = m
        antenv.axon_hooks = m
        return True
    except Exception:
        return False


def _build_program():
    import concourse.bacc as bacc
    import concourse.tile as tile
    from concourse import bass, mybir

    f32 = mybir.dt.float32
    bf16 = mybir.dt.bfloat16
    fp8 = mybir.dt.float8e4
    i32 = mybir.dt.int32
    RELU = mybir.ActivationFunctionType.Relu

    nc = bacc.Bacc("TRN2", target_bir_lowering=False, debug=False)

    ids_ap = nc.dram_tensor("idsT", [128, NCH], i32, kind="ExternalInput").ap()
    we_ap = nc.dram_tensor("web", [V, H], bf16, kind="ExternalInput").ap()
    # w1ee packs the K-augmented operands transposed: DRAM [MH_PAD+128, 128]
    # where col k<=100 row m = W1aug[k, m] (row 100 = b1|ones, rows 101..127
    # zero). Loaded with DMA-transpose: the M2S side reads the whole tensor
    # contiguously (fat descriptors that win the SDMA round-robin against the
    # gather queue's 1536B descriptors), the xbar writes SBUF [128, 1152].
    MH_PAD = 128 * KCH
    WEE_F = MH_PAD + 128  # 1024 cols of W1 + 32 of eeT + 96 pad
    w1ee_ap = nc.dram_tensor("w1eeT", [WEE_F, 128], bf16, kind="ExternalInput").ap()
    # w2p packs bf16 chunk-major: [128, KCH*H], column block k*H:(k+1)*H = k-chunk
    w2_ap = nc.dram_tensor("w2p", [128, KCH * H], bf16, kind="ExternalInput").ap()
    # b2o [1, H+NE]: [b2 | ones] -> K=1 bias matmul
    b2o_ap = nc.dram_tensor("b2o", [1, H + NE], bf16, kind="ExternalInput").ap()
    maskT_ap = nc.dram_tensor("maskT", [NE, TOK], bf16, kind="ExternalInput").ap()
    out_ap = nc.dram_tensor("out", [TOK, H], bf16, kind="ExternalOutput").ap()

    with tile.TileContext(nc) as tc, ExitStack() as ctx:
        const = ctx.enter_context(tc.tile_pool(name="const", bufs=1))
        psA = ctx.enter_context(tc.tile_pool(name="psA", bufs=2, space="PSUM"))
        psB = ctx.enter_context(tc.tile_pool(name="psB", bufs=1, space="PSUM"))
        psC = ctx.enter_context(tc.tile_pool(name="psC", bufs=2, space="PSUM"))
        gpool = ctx.enter_context(tc.tile_pool(name="gath", bufs=NCH))
        opool = ctx.enter_context(tc.tile_pool(name="outp", bufs=14))

        # ---- loads. The gather descriptor generation on gpsimd (Q7) costs
        # ~1.4us per 128-row indirect DMA (emission-paced; multi-column
        # offset APs and dma_gather both fail on HW), so the 16 gathers are
        # the kernel's pacer. ids + w1ee + w2 ride the sync HWDGE queue in
        # priority order (q10/scalar gets starved under SDMA round-robin
        # contention, so nothing MLP-critical goes there).
        ids_sb = const.tile([128, NCH], i32)
        nc.sync.dma_start(ids_sb[:], ids_ap[:])
        w1ee_sb = const.tile([128, WEE_F], bf16)
        nc.sync.dma_start(w1ee_sb[:], w1ee_ap[:], transpose=True)
        w1_sb = w1ee_sb[:, :MH_PAD]
        ee_sb = w1ee_sb[:, MH_PAD : MH_PAD + NE]
        KQ = KCH // 2
        w2_q = []
        for q in range(2):
            wq = const.tile([128, KQ * H], bf16, tag=f"w2q{q}")
            # Half 1 rides the gpsimd queue ahead of the gathers: its
            # descriptor generation overlaps the otherwise-idle wait for ids
            # and the data lands early at the fat-descriptor rate. Half 0
            # stays on sync HWDGE (moving both halves to gpsimd slows the
            # gather drain cadence and measures worse).
            w2eng = nc.sync if q == 0 else nc.gpsimd
            w2eng.dma_start(wq[:], w2_ap[:, q * KQ * H : (q + 1) * KQ * H])
            w2_q.append(wq)
        b2o_sb = const.tile([1, H + NE], bf16)
        nc.scalar.dma_start(b2o_sb[:], b2o_ap[:])
        b2_row = b2o_sb[:, :H]
        ones_row = b2o_sb[:, H : H + NE]
        maskT_sb = const.tile([NE, TOK], bf16)
        nc.scalar.dma_start(maskT_sb[:], maskT_ap[:])
        gts = []
        for g in range(NCH):
            gt = gpool.tile([128, H], bf16)
            nc.gpsimd.indirect_dma_start(
                out=gt[:],
                out_offset=None,
                in_=we_ap[:],
                in_offset=bass.IndirectOffsetOnAxis(ap=ids_sb[:, g : g + 1], axis=0),
            )
            gts.append(gt)

        # ---- MLP stage 1: hT[k*128+p, e] = relu(W1.T @ ee.T + b1) ----------
        # K=101 contraction: the b1|ones row folds the bias in, so the ACT
        # relu has no bias-column dependency and writes bf16 directly.
        # Chunks are paired per PSUM tile (2 matmuls, 1 ACTIVATE) to halve
        # the psA round-trips with only 2 PSUM banks.
        hT = const.tile([128, KCH, NE], bf16)
        # pairs 2,3 (hT chunks 4..7) first: MLP2 consumes those k-chunks
        # first (their w2 half lands early), so they must exist earliest.
        for kp in (2, 3, 0, 1):
            ps = psA.tile([128, 2, NE], f32, tag="ps")
            for i in range(2):
                k = 2 * kp + i
                nc.tensor.matmul(
                    out=ps[:, i, :],
                    lhsT=w1_sb[:, k * 128 : (k + 1) * 128],
                    rhs=ee_sb[:],
                    start=True,
                    stop=True,
                )
            nc.scalar.activation(
                out=hT[:, 2 * kp : 2 * kp + 2, :], in_=ps[:], func=RELU
            )

        # ---- MLP stage 2: ent = hT.T @ W2 + b2 ------------------------------
        # b2 enters the PSUM accumulation as a K=1 matmul of ones.T @ b2,
        # ordered LAST so the tiny b2o load never gates the k-loop.
        entp = psB.tile([NE, H], f32)
        NGROUPS = ((0, 512), (512, H))
        # k-chunks 4..7 accumulate first (their w2 half rides gpsimd and
        # lands early); only chunks 0..3 wait for the sync-queue half.
        korder = list(range(KQ, KCH)) + list(range(KQ))
        for idx, k in enumerate(korder):
            wq = w2_q[k // KQ]
            koff = (k % KQ) * H
            for n0, n1 in NGROUPS:
                nc.tensor.matmul(
                    out=entp[:, n0:n1],
                    lhsT=hT[:, k, :],
                    rhs=wq[:, koff + n0 : koff + n1],
                    start=(idx == 0),
                    stop=False,
                )
        for n0, n1 in NGROUPS:
            nc.tensor.matmul(
                out=entp[:, n0:n1],
                lhsT=ones_row[:],
                rhs=b2_row[:, n0:n1],
                start=False,
                stop=True,
            )
        ent_sb = const.tile([NE, H], bf16)
        # cast per n-group so the first scatter matmul (reads cols 0:512)
        # starts before the tail columns are cast
        for n0, n1 in NGROUPS:
            nc.scalar.copy(ent_sb[:, n0:n1], entp[:, n0:n1])

        # ---- main loop: scatter-matmul, add, store -------------------------
        # Plain 1x PSUM-operand adds on DVE. (Splitting the add through an
        # ACT psum->bf16 copy + 2x bf16 DVE add measures ~10us WORSE: the
        # ACT engine shares its sequencer with the scalar-queue HWDGE
        # descriptor generation, so loading it with copies serializes the
        # odd-chunk store issue behind the copy stream.)
        for g in range(NCH):
            gt = gts[g]
            sc = psC.tile([128, H], f32)
            for n0, n1 in NGROUPS:
                nc.tensor.matmul(
                    out=sc[:, n0:n1],
                    lhsT=maskT_sb[:, g * 128 : (g + 1) * 128],
                    rhs=ent_sb[:, n0:n1],
                    start=True,
                    stop=True,
                )
            ot = opool.tile([128, H], bf16)
            nc.vector.tensor_add(ot[:], gt[:], sc[:])
            st_eng = nc.sync if g % 2 == 0 else nc.scalar
            st_eng.dma_start(out_ap[g * 128 : (g + 1) * 128, :], ot[:])

    nc.compile()
    return nc


def _get_program():
    global _PROGRAM
    if _PROGRAM is None:
        _PROGRAM = _build_program()
    return _PROGRAM


def _prep_shards(inputs):
    import ml_dtypes

    bf16 = ml_dtypes.bfloat16
    MH_PAD = 128 * KCH

    ids = np.ascontiguousarray(np.asarray(inputs["input_ids"]).astype(np.int32))
    ee = np.asarray(inputs["entity_embeddings"], dtype=np.float32)
    mask = np.asarray(inputs["entity_mask"], dtype=np.float32)
    web = np.ascontiguousarray(
        np.asarray(inputs["word_embedding"], dtype=np.float32).astype(bf16)
    )
    W1 = np.asarray(inputs["W1"], dtype=np.float32)
    b1 = np.asarray(inputs["b1"], dtype=np.float32)
    W2 = np.asarray(inputs["W2"], dtype=np.float32)
    b2 = np.asarray(inputs["b2"], dtype=np.float32)

    WEE_F = MH_PAD + 128
    # K-augmented [128, WEE_F]: rows 0..99 = W1 | eeT, row 100 = b1 | ones,
    # rows 101..127 zero (so MLP1 contracts a uniform K=128).
    w1aug = np.zeros((128, WEE_F), np.float32)
    w1aug[:KG, :MH] = W1
    w1aug[KG, :MH] = b1  # b1 row: contracted against the ones row in ee
    w1aug[KG, MH_PAD : MH_PAD + NE] = 1.0  # ones row for eeT
    w2_pad = np.concatenate([W2, np.zeros((MH_PAD - MH, H), np.float32)], 0)
    w2p = np.ascontiguousarray(
        w2_pad.reshape(KCH, 128, H).transpose(1, 0, 2).reshape(128, KCH * H).astype(bf16)
    )
    b2o = np.ascontiguousarray(
        np.concatenate([b2[None, :], np.ones((1, NE), np.float32)], 1).astype(bf16)
    )  # [1, H+NE]

    in_maps = []
    for i in range(NCORES):
        sl = slice(BPC * i, BPC * (i + 1))
        ids_i = ids[sl].reshape(-1)  # [TOK]
        idsT = np.ascontiguousarray(ids_i.reshape(NCH, 128).T)  # [128, NCH]
        w1aug_i = w1aug.copy()
        w1aug_i[:KG, MH_PAD : MH_PAD + NE] = ee[sl].reshape(NE, KG).T
        w1eeT = np.ascontiguousarray(w1aug_i.T.astype(bf16))  # [WEE_F, 128]
        # block-diagonal [NE, TOK] mask (0/1 values: exact in bf16)
        maskT = np.zeros((NE, TOK), np.float32)
        for b in range(BPC):
            maskT[b * E : (b + 1) * E, b * S : (b + 1) * S] = mask[BPC * i + b]
        in_maps.append(
            {
                "idsT": idsT,
                "web": web,
                "w1eeT": w1eeT,
                "w2p": w2p,
                "b2o": b2o,
                "maskT": np.ascontiguousarray(maskT.astype(bf16)),
            }
        )
    return in_maps


def kernel(**inputs) -> np.ndarray:
    from concourse.bass_utils import run_bass_kernel_spmd

    trace = _maybe_enable_profiling()
    nc = _get_program()
    in_maps = _prep_shards(inputs)
    res = run_bass_kernel_spmd(
        nc, in_maps, core_ids=list(range(NCORES)), trace=trace
    )
    if trace and res.exec_time_ns is not None:
        print(f"HW exec time: {res.exec_time_ns} ns")
    out = np.concatenate(
        [
            res.results[i]["out"].astype(np.float32).reshape(BPC, S, H)
            for i in range(NCORES)
        ],
        0,
    )
    return out


if __name__ == "__main__":
    rng = np.random.default_rng(0)
    inputs = {
        "input_ids": rng.integers(0, V, (B, S)).astype(np.int32),
        "entity_embeddings": rng.standard_normal((B, E, KG), dtype=np.float32),
        "entity_mask": (rng.random((B, E, S)) < 0.02).astype(np.float32),
        "word_embedding": rng.standard_normal((V, H), dtype=np.float32) * 0.02,
        "W1": rng.standard_normal((KG, MH), dtype=np.float32) * 0.02,
        "b1": np.zeros(MH, np.float32),
        "W2": rng.standard_normal((MH, H), dtype=np.float32) * 0.02,
        "b2": np.zeros(H, np.float32),
    }
    out = kernel(**inputs)
    ref = inputs["word_embedding"][inputs["input_ids"]] + np.einsum(
        "bes,beh->bsh",
        inputs["entity_mask"],
        np.maximum(
            inputs["entity_embeddings"] @ inputs["W1"] + inputs["b1"], 0.0
        )
        @ inputs["W2"]
        + inputs["b2"],
    )
    err = np.abs(out - ref).max() / max(np.abs(ref).max(), 1e-12)
    print("self-check rel err:", err)


# revision 41
# speedup vs baseline: 1.1031x; 1.1031x over previous
"""Trainium2 Bass kernel for KG-enhanced embedding model (gnn_message_passing).

Computes, for full inputs:
    inputs_embeds = word_embedding[input_ids]                       # [B,S,H] gather
    h   = relu(entity_embeddings @ W1 + b1)                         # [B,E,MLP_HID]
    ent = h @ W2 + b2                                               # [B,E,H]
    out = inputs_embeds + einsum('bes,beh->bsh', entity_mask, ent)  # masked scatter-add

Sharding: data-parallel over batch B=32 -> 4 examples per core on 8 cores.
Weights and the vocab table are replicated; the gather reads only the rows
each core needs via indirect DMA (16 x 128-row gathers per core).

Numerics: the accuracy gate is rel_err < 2e-2, so everything runs in plain
bf16 (vocab table pre-cast to bf16 on host, bf16 matmuls with f32 PSUM
accumulation, bf16 output store upcast to f32 on host). Measured end-to-end
relative error ~5.8e-3. (fp8 e4m3 for the gather table also passes at
7.3e-3 but measured slower: the gather is descriptor-emission-paced, not
byte-paced, so halving row bytes buys nothing and starves the weights.)

Schedule notes (TRN2-specific, from perfetto traces):
- The per-instruction SWDGE descriptor generation on gpsimd (~1.4us per
  128-row indirect DMA, emission-paced) is the kernel's pacer: 16 x 1.4us
  is the wall everything else must hide behind. ids ride the sync HWDGE
  so gather desc-gen starts right after the framework preamble; W2's
  second half rides the gpsimd queue ahead of the gathers (its desc-gen
  overlaps the idle ids wait, its data lands before the contention).
- Under SDMA round-robin contention, a queue's bandwidth share scales
  with its descriptor size, so weight loads must use fat descriptors:
  W1 is loaded via DMA-transpose (contiguous M2S reads), W2 in two
  [128, 4*768] chunks (6KB/partition descriptors). Nothing MLP-critical
  goes on the scalar queue (it gets starved during the gather window).
- b1 is folded into the MLP1 contraction (K padded to 128 with a b1|ones
  row), b2 enters as a trailing K=1 ones-matmul: no bias-load can stall
  the MLP, which must finish before the add/store tail starts.

Shapes (hardcoded): V=30522, H=768, B=32, S=512, E=8, KG=100, MH=1000.
"""

import os
import numpy as np
from contextlib import ExitStack

V, H = 30522, 768
B, S, E = 32, 512, 8
KG, MH = 100, 1000
NCORES = 8
BPC = B // NCORES              # examples per core = 4
TOK = BPC * S                  # tokens per core = 2048
NCH = TOK // 128               # 128-token chunks per core = 16
KCH = 8                        # K chunks of 128 for the 1024-dim (padded) contraction
NE = BPC * E                   # entities per core = 32

_PROGRAM = None


def _maybe_enable_profiling():
    """Optional NTFF profiling (KERNEL_PROFILE=1): shim antenv.axon_hooks."""
    if os.environ.get("KERNEL_PROFILE") != "1":
        return False
    import sys, types
    try:
        from antenv.axon_hooks import get_axon_ntff_profile_hook  # noqa: F401
        return True
    except ImportError:
        pass
    try:
        from trn_agent_boot.trn_boot import _ntff_profile_via_ctypes
        import antenv
        hook = _ntff_profile_via_ctypes("/opt/axon/libaxon_pjrt.so")
        m = types.ModuleType("antenv.axon_hooks")
        m.get_axon_ntff_profile_hook = lambda: hook
        m.set_axon_ntff_profile_hook = lambda h: None
        sys.modules["antenv.axon_hooks"] = m
        antenv.axon_hooks = m
        return True
    except Exception:
        return False


def _build_program():
    import concourse.bacc as bacc
    import concourse.tile as tile
    from concourse import bass, mybir

    f32 = mybir.dt.float32
    bf16 = mybir.dt.bfloat16
    fp8 = mybir.dt.float8e4
    i32 = mybir.dt.int32
    RELU = mybir.ActivationFunctionType.Relu

    nc = bacc.Bacc("TRN2", target_bir_lowering=False, debug=False)

    ids_ap = nc.dram_tensor("idsT", [128, NCH], i32, kind="ExternalInput").ap()
    we_ap = nc.dram_tensor("web", [V, H], bf16, kind="ExternalInput").ap()
    # w1ee packs the K-augmented operands transposed: DRAM [MH_PAD+128, 128]
    # where col k<=100 row m = W1aug[k, m] (row 100 = b1|ones, rows 101..127
    # zero). Loaded with DMA-transpose: the M2S side reads the whole tensor
    # contiguously (fat descriptors that win the SDMA round-robin against the
    # gather queue's 1536B descriptors), the xbar writes SBUF [128, 1152].
    MH_PAD = 128 * KCH
    WEE_F = MH_PAD + 128  # 1024 cols of W1 + 32 of eeT + 96 pad
    w1ee_ap = nc.dram_tensor("w1eeT", [WEE_F, 128], bf16, kind="ExternalInput").ap()
    # w2p packs bf16 chunk-major: [128, KCH*H], column block k*H:(k+1)*H = k-chunk
    w2_ap = nc.dram_tensor("w2p", [128, KCH * H], bf16, kind="ExternalInput").ap()
    # b2o [1, H+NE]: [b2 | ones] -> K=1 bias matmul
    b2o_ap = nc.dram_tensor("b2o", [1, H + NE], bf16, kind="ExternalInput").ap()
    maskT_ap = nc.dram_tensor("maskT", [NE, TOK], bf16, kind="ExternalInput").ap()
    out_ap = nc.dram_tensor("out", [TOK, H], bf16, kind="ExternalOutput").ap()

    with tile.TileContext(nc) as tc, ExitStack() as ctx:
        const = ctx.enter_context(tc.tile_pool(name="const", bufs=1))
        psA = ctx.enter_context(tc.tile_pool(name="psA", bufs=2, space="PSUM"))
        psB = ctx.enter_context(tc.tile_pool(name="psB", bufs=1, space="PSUM"))
        psC = ctx.enter_context(tc.tile_pool(name="psC", bufs=2, space="PSUM"))
        gpool = ctx.enter_context(tc.tile_pool(name="gath", bufs=NCH))
        opool = ctx.enter_context(tc.tile_pool(name="outp", bufs=14))

        # ---- loads. The gather descriptor generation on gpsimd (Q7) costs
        # ~1.4us per 128-row indirect DMA (emission-paced; multi-column
        # offset APs and dma_gather both fail on HW), so the 16 gathers are
        # the kernel's pacer. ids + w1ee + w2 ride the sync HWDGE queue in
        # priority order (q10/scalar gets starved under SDMA round-robin
        # contention, so nothing MLP-critical goes there).
        ids_sb = const.tile([128, NCH], i32)
        nc.sync.dma_start(ids_sb[:], ids_ap[:])
        w1ee_sb = const.tile([128, WEE_F], bf16)
        nc.sync.dma_start(w1ee_sb[:], w1ee_ap[:], transpose=True)
        w1_sb = w1ee_sb[:, :MH_PAD]
        ee_sb = w1ee_sb[:, MH_PAD : MH_PAD + NE]
        KQ = KCH // 2
        w2_q = []
        for q in range(2):
            wq = const.tile([128, KQ * H], bf16, tag=f"w2q{q}")
            # Half 1 rides the gpsimd queue ahead of the gathers: its
            # descriptor generation overlaps the otherwise-idle wait for ids
            # and the data lands early at the fat-descriptor rate. Half 0
            # stays on sync HWDGE (moving both halves to gpsimd slows the
            # gather drain cadence and measures worse).
            w2eng = nc.sync if q == 0 else nc.gpsimd
            w2eng.dma_start(wq[:], w2_ap[:, q * KQ * H : (q + 1) * KQ * H])
            w2_q.append(wq)
        b2o_sb = const.tile([1, H + NE], bf16)
        nc.scalar.dma_start(b2o_sb[:], b2o_ap[:])
        b2_row = b2o_sb[:, :H]
        ones_row = b2o_sb[:, H : H + NE]
        maskT_sb = const.tile([NE, TOK], bf16)
        nc.scalar.dma_start(maskT_sb[:], maskT_ap[:])
        gts = []
        for g in range(NCH):
            gt = gpool.tile([128, H], bf16)
            nc.gpsimd.indirect_dma_start(
                out=gt[:],
                out_offset=None,
                in_=we_ap[:],
                in_offset=bass.IndirectOffsetOnAxis(ap=ids_sb[:, g : g + 1], axis=0),
            )
            gts.append(gt)

        # ---- MLP stage 1: hT[k*128+p, e] = relu(W1.T @ ee.T + b1) ----------
        # K=101 contraction: the b1|ones row folds the bias in, so the ACT
        # relu has no bias-column dependency and writes bf16 directly.
        # Chunks are paired per PSUM tile (2 matmuls, 1 ACTIVATE) to halve
        # the psA round-trips with only 2 PSUM banks.
        hT = const.tile([128, KCH, NE], bf16)
        # pairs 2,3 (hT chunks 4..7) first: MLP2 consumes those k-chunks
        # first (their w2 half lands early), so they must exist earliest.
        for kp in (2, 3, 0, 1):
            ps = psA.tile([128, 2, NE], f32, tag="ps")
            for i in range(2):
                k = 2 * kp + i
                nc.tensor.matmul(
                    out=ps[:, i, :],
                    lhsT=w1_sb[:, k * 128 : (k + 1) * 128],
                    rhs=ee_sb[:],
                    start=True,
                    stop=True,
                )
            nc.scalar.activation(
                out=hT[:, 2 * kp : 2 * kp + 2, :], in_=ps[:], func=RELU
            )

        # ---- MLP stage 2: ent = hT.T @ W2 + b2 ------------------------------
        # b2 enters the PSUM accumulation as a K=1 matmul of ones.T @ b2,
        # ordered LAST so the tiny b2o load never gates the k-loop.
        entp = psB.tile([NE, H], f32)
        NGROUPS = ((0, 512), (512, H))
        # k-chunks 4..7 accumulate first (their w2 half rides gpsimd and
        # lands early); only chunks 0..3 wait for the sync-queue half.
        korder = list(range(KQ, KCH)) + list(range(KQ))
        for idx, k in enumerate(korder):
            wq = w2_q[k // KQ]
            koff = (k % KQ) * H
            for n0, n1 in NGROUPS:
                nc.tensor.matmul(
                    out=entp[:, n0:n1],
                    lhsT=hT[:, k, :],
                    rhs=wq[:, koff + n0 : koff + n1],
                    start=(idx == 0),
                    stop=False,
                )
        for n0, n1 in NGROUPS:
            nc.tensor.matmul(
                out=entp[:, n0:n1],
                lhsT=ones_row[:],
                rhs=b2_row[:, n0:n1],
                start=False,
                stop=True,
            )
        ent_sb = const.tile([NE, H], bf16)
        nc.scalar.copy(ent_sb[:], entp[:])  # cast f32 -> bf16

        # ---- main loop: scatter-matmul, add, store -------------------------
        # Plain 1x PSUM-operand adds on DVE. (Splitting the add through an
        # ACT psum->bf16 copy + 2x bf16 DVE add measures ~10us WORSE: the
        # ACT engine shares its sequencer with the scalar-queue HWDGE
        # descriptor generation, so loading it with copies serializes the
        # odd-chunk store issue behind the copy stream.)
        for g in range(NCH):
            gt = gts[g]
            sc = psC.tile([128, H], f32)
            for n0, n1 in NGROUPS:
                nc.tensor.matmul(
                    out=sc[:, n0:n1],
                    lhsT=maskT_sb[:, g * 128 : (g + 1) * 128],
                    rhs=ent_sb[:, n0:n1],
                    start=True,
                    stop=True,
                )
            ot = opool.tile([128, H], bf16)
            nc.vector.tensor_add(ot[:], gt[:], sc[:])
            st_eng = nc.sync if g % 2 == 0 else nc.scalar
            st_eng.dma_start(out_ap[g * 128 : (g + 1) * 128, :], ot[:])

    nc.compile()
    return nc


def _get_program():
    global _PROGRAM
    if _PROGRAM is None:
        _PROGRAM = _build_program()
    return _PROGRAM


def _prep_shards(inputs):
    import ml_dtypes

    bf16 = ml_dtypes.bfloat16
    MH_PAD = 128 * KCH

    ids = np.ascontiguousarray(np.asarray(inputs["input_ids"]).astype(np.int32))
    ee = np.asarray(inputs["entity_embeddings"], dtype=np.float32)
    mask = np.asarray(inputs["entity_mask"], dtype=np.float32)
    web = np.ascontiguousarray(
        np.asarray(inputs["word_embedding"], dtype=np.float32).astype(bf16)
    )
    W1 = np.asarray(inputs["W1"], dtype=np.float32)
    b1 = np.asarray(inputs["b1"], dtype=np.float32)
    W2 = np.asarray(inputs["W2"], dtype=np.float32)
    b2 = np.asarray(inputs["b2"], dtype=np.float32)

    WEE_F = MH_PAD + 128
    # K-augmented [128, WEE_F]: rows 0..99 = W1 | eeT, row 100 = b1 | ones,
    # rows 101..127 zero (so MLP1 contracts a uniform K=128).
    w1aug = np.zeros((128, WEE_F), np.float32)
    w1aug[:KG, :MH] = W1
    w1aug[KG, :MH] = b1  # b1 row: contracted against the ones row in ee
    w1aug[KG, MH_PAD : MH_PAD + NE] = 1.0  # ones row for eeT
    w2_pad = np.concatenate([W2, np.zeros((MH_PAD - MH, H), np.float32)], 0)
    w2p = np.ascontiguousarray(
        w2_pad.reshape(KCH, 128, H).transpose(1, 0, 2).reshape(128, KCH * H).astype(bf16)
    )
    b2o = np.ascontiguousarray(
        np.concatenate([b2[None, :], np.ones((1, NE), np.float32)], 1).astype(bf16)
    )  # [1, H+NE]

    in_maps = []
    for i in range(NCORES):
        sl = slice(BPC * i, BPC * (i + 1))
        ids_i = ids[sl].reshape(-1)  # [TOK]
        idsT = np.ascontiguousarray(ids_i.reshape(NCH, 128).T)  # [128, NCH]
        w1aug_i = w1aug.copy()
        w1aug_i[:KG, MH_PAD : MH_PAD + NE] = ee[sl].reshape(NE, KG).T
        w1eeT = np.ascontiguousarray(w1aug_i.T.astype(bf16))  # [WEE_F, 128]
        # block-diagonal [NE, TOK] mask (0/1 values: exact in bf16)
        maskT = np.zeros((NE, TOK), np.float32)
        for b in range(BPC):
            maskT[b * E : (b + 1) * E, b * S : (b + 1) * S] = mask[BPC * i + b]
        in_maps.append(
            {
                "idsT": idsT,
                "web": web,
                "w1eeT": w1eeT,
                "w2p": w2p,
                "b2o": b2o,
                "maskT": np.ascontiguousarray(maskT.astype(bf16)),
            }
        )
    return in_maps


def kernel(**inputs) -> np.ndarray:
    from concourse.bass_utils import run_bass_kernel_spmd

    trace = _maybe_enable_profiling()
    nc = _get_program()
    in_maps = _prep_shards(inputs)
    res = run_bass_kernel_spmd(
        nc, in_maps, core_ids=list(range(NCORES)), trace=trace
    )
    if trace and res.exec_time_ns is not None:
        print(f"HW exec time: {res.exec_time_ns} ns")
    out = np.concatenate(
        [
            res.results[i]["out"].astype(np.float32).reshape(BPC, S, H)
            for i in range(NCORES)
        ],
        0,
    )
    return out


if __name__ == "__main__":
    rng = np.random.default_rng(0)
    inputs = {
        "input_ids": rng.integers(0, V, (B, S)).astype(np.int32),
        "entity_embeddings": rng.standard_normal((B, E, KG), dtype=np.float32),
        "entity_mask": (rng.random((B, E, S)) < 0.02).astype(np.float32),
        "word_embedding": rng.standard_normal((V, H), dtype=np.float32) * 0.02,
        "W1": rng.standard_normal((KG, MH), dtype=np.float32) * 0.02,
        "b1": np.zeros(MH, np.float32),
        "W2": rng.standard_normal((MH, H), dtype=np.float32) * 0.02,
        "b2": np.zeros(H, np.float32),
    }
    out = kernel(**inputs)
    ref = inputs["word_embedding"][inputs["input_ids"]] + np.einsum(
        "bes,beh->bsh",
        inputs["entity_mask"],
        np.maximum(
            inputs["entity_embeddings"] @ inputs["W1"] + inputs["b1"], 0.0
        )
        @ inputs["W2"]
        + inputs["b2"],
    )
    err = np.abs(out - ref).max() / max(np.abs(ref).max(), 1e-12)
    print("self-check rel err:", err)
